# revision 1
# baseline (speedup 1.0000x reference)
"""Trainium2 kernel for nn_Activation1d (BigVGAN up->SnakeBeta->down), 8-core SPMD.

Math (per row; a=exp(alpha), invb2=1/(2*(exp(beta)+1e-9)) per channel):
    out = G@x + invb2 + invb2 * (D0@c0 + D1@c1),  c_p = -cos(2*a*y_p)
where y_p are the two phases of the 2x-upsampled signal and U_p, G, D_p are
banded T x T operators (replicate padding folded in).

On-chip realization (t-major [t partitions, seq free] via PE transposes):
    xa   = x * (a/pi)                     (seq-major, per-partition scalar)
    xa^T = PE transpose                   (the only transposed input)
    r'_p = U_p @ xa^T + 0.25              (PSUM; == u/(2pi) + 1/4)
    n    = (r' + MAGIC) - MAGIC           (round-to-nearest, fp32 trick)
    w    = r' - n                         (in [-0.5, 0.5])
    c_p  = Sin(-2pi * w)                  (== -cos(u), ACT table domain safe)
    P1   = G @ xa^T + (a*invb2/pi)x1      (rank-1 via ones x row matmul)
    P2   = D0@c0 + D1@c1
    out^T = P1*(pi/a) + P2*invb2          (broadcast tiles, per-seq-column)
    out  = PE transpose back, copy, DMA

All five operators are applied as 128x128 block matmuls with prev/main/next
(+ boundary-fixed first/last) band matrices, precomputed on host by probing
a pure-numpy port of the reference on an identity batch.
"""
import math
import numpy as np

import concourse.bass as bass
import concourse.bacc as bacc
import concourse.mybir as mybir
from concourse.tile import TileContext
from concourse.bass_utils import run_bass_kernel_spmd

F32 = mybir.dt.float32
F32R = mybir.dt.float32r
BF16 = mybir.dt.bfloat16

MMDT = 'f32r'  # matmul operand dtype: 'f32' | 'f32r' | 'bf16'
MDT = {'f32': F32, 'f32r': F32R, 'bf16': BF16}[MMDT]

B_, C_, T_ = 16, 256, 8192
K_ = 12
NCORES = 8
ROWS_PER_CORE = (B_ * C_) // NCORES   # 512
NB = T_ // 128                        # 64 blocks of 128 t
GRP = 8                               # t-blocks per DMA super-tile
NGRP = NB // GRP


# ---------------------------------------------------------------- host math
def _np_up_linop(x, up_filter):
    """[N, T] -> [N, 2T]; exact port of reference UpSample1d (numpy only)."""
    n, t = x.shape
    pad = K_ // 2 - 1                       # 5
    pad_left = pad * 2 + (K_ - 2) // 2      # 15
    xp = np.concatenate([np.repeat(x[:, :1], pad, 1), x,
                         np.repeat(x[:, -1:], pad, 1)], axis=1)
    L = t + 2 * pad
    xd = np.zeros((n, 2 * L - 1), dtype=np.float32)
    xd[:, ::2] = xp
    yf = np.zeros((n, 2 * L - 1 + K_ - 1), dtype=np.float32)
    for m in range(K_):
        yf[:, m:m + 2 * L - 1] += np.float32(up_filter[m]) * xd
    return (2.0 * yf[:, pad_left:pad_left + 2 * t]).astype(np.float32)


def _np_down_linop(z, down_filter):
    """[N, 2T] -> [N, T]; exact port of reference DownSample1d."""
    n, t2 = z.shape
    dpl, dpr = K_ // 2 - 1, K_ // 2         # 5, 6
    zp = np.concatenate([np.repeat(z[:, :1], dpl, 1), z,
                         np.repeat(z[:, -1:], dpr, 1)], axis=1)
    t = t2 // 2
    out = np.zeros((n, t), dtype=np.float32)
    for j in range(K_):
        out += np.float32(down_filter[j]) * zp[:, j:j + 2 * t - 1:2]
    return out


def _extract_blocks(up_filter, down_filter, Tp=512, B=128):
    I = np.eye(Tp, dtype=np.float32)
    Ufull = _np_up_linop(I, up_filter).T            # [2Tp, Tp]
    I2 = np.eye(2 * Tp, dtype=np.float32)
    Dfull = _np_down_linop(I2, down_filter).T       # [Tp, 2Tp]
    U0, U1 = Ufull[0::2, :], Ufull[1::2, :]
    G = (Dfull.astype(np.float64) @ Ufull.astype(np.float64)).astype(np.float32)
    D0, D1 = Dfull[:, 0::2].copy(), Dfull[:, 1::2].copy()
    nb = Tp // B
    rmid = nb // 2
    blocks = {}
    for name, A in (('U0', U0), ('U1', U1), ('G', G), ('D0', D0), ('D1', D1)):
        blocks[name] = dict(
            main=A[rmid*B:(rmid+1)*B, rmid*B:(rmid+1)*B].copy(),
            next=A[rmid*B:(rmid+1)*B, (rmid+1)*B:(rmid+2)*B].copy(),
            prev=A[rmid*B:(rmid+1)*B, (rmid-1)*B:rmid*B].copy(),
            first_main=A[0:B, 0:B].copy(),
            last_main=A[(nb-1)*B:, (nb-1)*B:].copy(),
        )
    return blocks


# band order inside the packed const tensor
_BAND_NAMES = []
for _op in ('U0', 'U1', 'G', 'D0', 'D1'):
    for _v in ('main', 'next', 'prev', 'first_main', 'last_main'):
        _BAND_NAMES.append((_op, _v))
_BIDX = {k: i for i, k in enumerate(_BAND_NAMES)}
NBANDS = len(_BAND_NAMES)

# packed fp32 const layout offsets
_CW_IDENT = 0
_CW_INVB2 = 128
_CW_PIAT = 640
_CW_A2 = 1152
_CW_ZERO = 1156
CW = 1157
# packed MDT const rows after the bands: ainvb2pi | quarter | ones
MW = NBANDS * 128 + 512 + 512 + 128


# ---------------------------------------------------------------- bass kernel
def _build_bass():
    nc = bacc.Bacc()
    xs = nc.dram_tensor("xs", [ROWS_PER_CORE, T_], F32, kind="ExternalInput")
    cpack_d = nc.dram_tensor("cpack", [128, CW], F32, kind="ExternalInput")
    mpack_d = nc.dram_tensor("mpack", [128, MW], MDT, kind="ExternalInput")
    outs = nc.dram_tensor("outs", [ROWS_PER_CORE, T_], F32, kind="ExternalOutput")

    SB = 4          # seq blocks of 128 (512 rows)
    GW = GRP * 128  # group width in t (1024)
    MAGIC = 12582912.0   # 1.5 * 2**23
    TWOPI = 2.0 * math.pi

    with TileContext(nc) as tc:
        with (
            tc.tile_pool(name="const", bufs=1) as pconst,
            tc.tile_pool(name="sup", bufs=2) as psup,
            tc.tile_pool(name="sup1", bufs=1) as psup1,
            tc.tile_pool(name="xt", bufs=4) as pxt,
            tc.tile_pool(name="c", bufs=5) as pc,
            tc.tile_pool(name="mid", bufs=2) as pmid,
            tc.tile_pool(name="tp", bufs=2, space="PSUM") as ptp,
            tc.tile_pool(name="u", bufs=1, space="PSUM") as pu,
            tc.tile_pool(name="pp", bufs=1, space="PSUM") as ppp,
            tc.tile_pool(name="os", bufs=1, space="PSUM") as pos,
        ):
            cpack_s = pconst.tile([128, CW], F32, tag="cpack_s")
            nc.sync.dma_start(out=cpack_s[:], in_=cpack_d[:])
            mpack_s = pconst.tile([128, MW], MDT, tag="mpack_s")
            nc.sync.dma_start(out=mpack_s[:], in_=mpack_d[:])
            # stage through one DVE tick so every const consumer waits on DVE
            # only (ISA instructions support very few inline sync waits)
            mpack = pconst.tile([128, MW], MDT, tag="mpack")
            nc.vector.tensor_copy(mpack[:], mpack_s[:])
            cpack = pconst.tile([128, CW], F32, tag="cpack")
            nc.vector.tensor_copy(cpack[:], cpack_s[:])
            ident = cpack[:, _CW_IDENT:_CW_IDENT+128]
            invb2t = cpack[:, _CW_INVB2:_CW_INVB2+512]
            piat = cpack[:, _CW_PIAT:_CW_PIAT+512]
            a2t = cpack[:, _CW_A2:_CW_A2+4]
            mpic = cpack[:, _CW_ZERO:_CW_ZERO+1]   # -pi bias column
            band_t = mpack
            ainvb2pi = mpack[0:1, NBANDS*128:NBANDS*128+512]
            quarter = mpack[0:1, NBANDS*128+512:NBANDS*128+1024]
            ones1 = mpack[0:1, NBANDS*128+1024:NBANDS*128+1152]

            def band(op, which):
                i = _BIDX[(op, which)]
                return band_t[:, i*128:(i+1)*128]

            def bsel(op, b):
                if b == 0:
                    return band(op, 'first_main')
                if b == NB - 1:
                    return band(op, 'last_main')
                return band(op, 'main')

            xasup = {}    # group -> [128, SB*GW] (free = (sb, t_local))
            xat = {}      # block -> [128, 512] t-major (x * a/pi, transposed)
            cts = {}      # block -> (c0, c1)
            osup = {}     # group -> output super tile

            # PE warmup: one transpose so the PE has observed the const-staging
            # DVE tick before the first real transpose (1-wait ISA limit).
            warm = pos.tile([128, 128], F32, tag="os")
            nc.tensor.transpose(warm[:], ident, ident)

            xs_v = xs.rearrange("(sb s) (g t) -> sb s g t", s=128, t=GW)
            outs_v = outs.rearrange("(sb s) (g t) -> sb s g t", s=128, t=GW)

            def load_group(g):
                xg = psup1.tile([128, SB * GW], F32, tag="xs")
                nc.sync.dma_start(
                    out=xg.rearrange("s (sb t) -> s sb t", sb=SB),
                    in_=xs_v[:, :, g, :].rearrange("sb s t -> s sb t"))
                xa = psup.tile([128, SB * GW], F32, tag="xa")
                for sb in range(SB):
                    nc.vector.tensor_scalar(
                        out=xa[:, sb*GW:(sb+1)*GW], in0=xg[:, sb*GW:(sb+1)*GW],
                        scalar1=a2t[:, sb:sb+1], scalar2=None,
                        op0=mybir.AluOpType.mult)
                xasup[g] = xa

            def transpose_block(b):
                g, tb = divmod(b, GRP)
                xa = xasup[g]
                xatp = ptp.tile([128, 512], F32, tag="xatp")
                for sb in range(SB):
                    sl = slice(sb*GW + tb*128, sb*GW + (tb+1)*128)
                    nc.tensor.transpose(xatp[:, sb*128:(sb+1)*128], xa[:, sl],
                                        ident)
                xats = pxt.tile([128, 512], MDT, tag="xat")
                nc.scalar.copy(xats[:], xatp[:])
                xat[b] = xats

            def up_block(b):
                c0 = pc.tile([128, 512], MDT, tag="c0")
                c1 = pc.tile([128, 512], MDT, tag="c1")
                for p, (op, ct) in enumerate((('U0', c0), ('U1', c1))):
                    ups = pu.tile([128, 512], F32, tag=f"u{p}")
                    mms = [(bsel(op, b), xat[b])]
                    if b > 0:
                        mms.append((band(op, 'prev'), xat[b-1]))
                    if b + 1 < NB:
                        mms.append((band(op, 'next'), xat[b+1]))
                    for i, (w, rhs) in enumerate(mms):
                        nc.tensor.matmul(ups[:], w, rhs[:],
                                         start=(i == 0), stop=False)
                    # ups = r' = u/2pi + 1/4;  n = round(r') via the fp32
                    # magic-number trick;  sin(2pi*(r'-n)) = cos(u)
                    # (D bands negated host-side absorb the -cos sign).
                    nc.tensor.matmul(ups[:], ones1, quarter,
                                     start=False, stop=True)
                    nt = pmid.tile([128, 512], F32, tag=f"n{p}")
                    nc.vector.tensor_scalar(
                        out=nt[:], in0=ups[:], scalar1=MAGIC, scalar2=MAGIC,
                        op0=mybir.AluOpType.add, op1=mybir.AluOpType.subtract)
                    wt = pmid.tile([128, 512], F32, tag=f"w{p}")
                    nc.vector.scalar_tensor_tensor(
                        out=wt[:], in0=nt[:], scalar=-1.0, in1=ups[:],
                        op0=mybir.AluOpType.mult, op1=mybir.AluOpType.add)
                    nc.scalar.activation(ct[:], wt[:],
                                         mybir.ActivationFunctionType.Sin,
                                         bias=mpic, scale=TWOPI)
                cts[b] = (c0, c1)

            def out_block(b):
                g, tb = divmod(b, GRP)
                p1 = ppp.tile([128, 512], F32, tag="p1")
                mms = [(bsel('G', b), xat[b])]
                if b > 0:
                    mms.append((band('G', 'prev'), xat[b-1]))
                if b + 1 < NB:
                    mms.append((band('G', 'next'), xat[b+1]))
                for i, (w, rhs) in enumerate(mms):
                    nc.tensor.matmul(p1[:], w, rhs[:],
                                     start=(i == 0), stop=False)
                nc.tensor.matmul(p1[:], ones1, ainvb2pi,
                                 start=False, stop=True)

                p2 = ppp.tile([128, 512], F32, tag="p2")
                mms = []
                for p, op in enumerate(('D0', 'D1')):
                    mms.append((bsel(op, b), cts[b][p]))
                    if b > 0:
                        mms.append((band(op, 'prev'), cts[b-1][p]))
                    if b + 1 < NB:
                        mms.append((band(op, 'next'), cts[b+1][p]))
                for i, (w, rhs) in enumerate(mms):
                    nc.tensor.matmul(p2[:], w, rhs[:],
                                     start=(i == 0), stop=(i == len(mms)-1))

                tmp = pmid.tile([128, 512], F32, tag="tmp")
                nc.vector.tensor_tensor(out=tmp[:], in0=p2[:], in1=invb2t,
                                        op=mybir.AluOpType.mult)
                tmpa = pmid.tile([128, 512], F32, tag="tmpa")
                nc.vector.tensor_tensor(out=tmpa[:], in0=p1[:], in1=piat,
                                        op=mybir.AluOpType.mult)
                zsum = pmid.tile([128, 512], F32, tag="zsum")
                nc.gpsimd.tensor_tensor(out=zsum[:], in0=tmp[:], in1=tmpa[:],
                                        op=mybir.AluOpType.add)

                osps = pos.tile([128, 512], F32, tag="os")
                for sb in range(SB):
                    nc.tensor.transpose(osps[:, sb*128:(sb+1)*128],
                                        zsum[:, sb*128:(sb+1)*128], ident)
                if tb == 0:
                    og_new = psup.tile([128, SB * GW], F32, tag="osup")
                    osup[g] = og_new
                og = osup[g]
                nc.scalar.copy(
                    og.rearrange("s (sb t) -> s sb t", sb=SB)[:, :, tb*128:(tb+1)*128],
                    osps.rearrange("s (sb t) -> s sb t", sb=SB))
                if tb == GRP - 1:
                    nc.sync.dma_start(
                        out=outs_v[:, :, g, :].rearrange("sb s t -> s sb t"),
                        in_=og.rearrange("s (sb t) -> s sb t", sb=SB))
                    # WAR dummy: makes DVE observe the store's DMA-lane tick, so
                    # the slot-reuse writer two groups later needs no DMA wait.
                    nc.vector.memset(og[0:1, 0:1], 0.0)

            for i in range(NB + 2):
                if i < NB:
                    if i % GRP == 0:
                        load_group(i // GRP)
                    transpose_block(i)
                if 1 <= i <= NB:
                    up_block(i - 1)
                if 2 <= i:
                    out_block(i - 2)

    nc.compile()
    return nc


_NC_CACHE = {}


def host_consts(alpha, beta, up_filter, down_filter):
    alpha = np.asarray(alpha, dtype=np.float32)
    beta = np.asarray(beta, dtype=np.float32)
    up_filter = np.asarray(up_filter, dtype=np.float32)
    down_filter = np.asarray(down_filter, dtype=np.float32)
    blocks = _extract_blocks(up_filter, down_filter)
    for op in ('D0', 'D1'):
        for v in blocks[op]:
            blocks[op][v] = -blocks[op][v]
    band_arr = np.concatenate(
        [blocks[op][v].T.copy() for op, v in _BAND_NAMES], axis=1)  # [128, N*128] lhsT

    arow = np.exp(np.tile(alpha, B_ // NCORES * 2)[:ROWS_PER_CORE]).astype(np.float32)
    a2 = (arow / np.float32(math.pi)).astype(np.float32)     # x scale: a/pi
    invb2 = (0.5 / (np.exp(np.tile(beta, B_ // NCORES * 2)[:ROWS_PER_CORE]) + 1e-9)
             ).astype(np.float32)
    piat = (np.float32(math.pi) / arow).astype(np.float32)   # undo a/pi on G path
    ainvb2pi = (invb2 * a2).astype(np.float32)               # rank-1 row: +invb2 after piat
    a2t = a2.reshape(4, 128).T.copy()                        # [128, 4]
    invb2t = np.broadcast_to(invb2[None, :], (128, 512)).copy()
    piatt = np.broadcast_to(piat[None, :], (128, 512)).copy()
    ident = np.eye(128, dtype=np.float32)
    mpic = np.zeros((128, 1), dtype=np.float32)

    np_mdt = mybir.dt.np(MDT)
    cpack = np.concatenate([ident, invb2t, piatt, a2t, mpic], axis=1)
    assert cpack.shape[1] == CW
    mrow = np.zeros((128, 512 + 512 + 128), dtype=np.float32)
    mrow[0, :512] = ainvb2pi
    mrow[0, 512:1024] = 0.25
    mrow[0, 1024:1024+128] = 1.0
    mpack = np.concatenate([band_arr, mrow], axis=1).astype(np_mdt)
    assert mpack.shape[1] == MW
    return dict(cpack=cpack, mpack=mpack)


def kernel(x, alpha, beta, up_filter, down_filter):
    x = np.ascontiguousarray(np.asarray(x, dtype=np.float32))
    consts = host_consts(alpha, beta, up_filter, down_filter)

    key = 'nc'
    if key not in _NC_CACHE:
        _NC_CACHE[key] = _build_bass()
    nc = _NC_CACHE[key]

    rows = x.reshape(B_ * C_, T_)
    in_maps = []
    for k in range(NCORES):
        shard = np.ascontiguousarray(rows[k*ROWS_PER_CORE:(k+1)*ROWS_PER_CORE])
        in_maps.append(dict(xs=shard, **consts))

    res = run_bass_kernel_spmd(nc, in_maps, core_ids=list(range(NCORES)),
                               **_RUN_KW)
    out = np.concatenate([r["outs"] for r in res.results], axis=0)
    kernel.last_result = res
    return out.reshape(B_, C_, T_)


_RUN_KW = {}
kernel.last_result = None



# revision 13
# speedup vs baseline: 1.3505x; 1.3505x over previous
"""Trainium2 kernel for nn_Activation1d (BigVGAN up->SnakeBeta->down), 8-core SPMD.

Math (per row; a=exp(alpha), invb=1/(exp(beta)+1e-9) per channel):
    out = D @ (U@x + invb * sin^2(a * U@x))
where U (2x upsample) and D (2x downsample) are narrow-banded Toeplitz
operators with replicate-pad boundaries.

On-chip realization (shifted-window blocks, L=116 outputs per block, one
128-wide input window each -- bands are only +-6 wide so prev/main/next
halo matmuls are unnecessary):
    xut   = x window, t-major, via transposing DMA (HBM -> SBUF xbar)   [fp16]
    xat2  = xut * (a/pi)  (broadcast TT)                                [fp16]
    r01   = [U0 @ xat2 | U1 @ xat2]   (both phases, PSUM fp32)
    m01   = mod(r01 + 8, 2)           (range reduction; +8 keeps the
                                       argument positive so C-fmod ==
                                       python-mod)                      [fp16]
    s01   = Sin(pi*m01 - pi) = sin(pi * a*y/pi) = sin(a*y)              [bf16]
    q01   = s01 * (s01 * invb) = invb * sin^2(a*y)                      [bf16]
    P     = G @ xut + D0 @ q0 + D1 @ q1    (single PSUM accumulation,
                                            G = D@U probed in float64)
    out   = PE-transpose(P as bf16) -> og supertile -> HBM (bf16, host
            upconverts to fp32)

Weight variants first/int/last fold the replicate padding; probed
numerically from a pure-numpy port of the reference on an identity batch.
"""
import math
import numpy as np
import ml_dtypes

import concourse.bass as bass
import concourse.bacc as bacc
import concourse.mybir as mybir
from concourse.tile import TileContext
from concourse.bass_utils import run_bass_kernel_spmd

F32 = mybir.dt.float32
F16 = mybir.dt.float16
BF16 = mybir.dt.bfloat16

B_, C_, T_ = 16, 256, 8192
K_ = 12
NCORES = 8
ROWS = (B_ * C_) // NCORES            # 512
L_ = 116
NB = 71                               # 70 blocks of 116 + tail of 72
GRP = 8
NGRP = (NB + GRP - 1) // GRP          # 9 (last group has 7 blocks)
TP = 512                              # host probe domain

MAGIC16 = 3072.0   # 1.5 * 2**11: fp16 spacing 2.0 -> store rounds to even ints


# ---------------------------------------------------------------- host math
def _np_up_linop(x, up_filter):
    n, t = x.shape
    pad = K_ // 2 - 1
    pad_left = pad * 2 + (K_ - 2) // 2
    xp = np.concatenate([np.repeat(x[:, :1], pad, 1), x,
                         np.repeat(x[:, -1:], pad, 1)], axis=1)
    L = t + 2 * pad
    xd = np.zeros((n, 2 * L - 1), dtype=np.float32)
    xd[:, ::2] = xp
    yf = np.zeros((n, 2 * L - 1 + K_ - 1), dtype=np.float32)
    for m in range(K_):
        yf[:, m:m + 2 * L - 1] += np.float32(up_filter[m]) * xd
    return (2.0 * yf[:, pad_left:pad_left + 2 * t]).astype(np.float32)


def _np_down_linop(z, down_filter):
    n, t2 = z.shape
    dpl, dpr = K_ // 2 - 1, K_ // 2
    zp = np.concatenate([np.repeat(z[:, :1], dpl, 1), z,
                         np.repeat(z[:, -1:], dpr, 1)], axis=1)
    t = t2 // 2
    out = np.zeros((n, t), dtype=np.float32)
    for j in range(K_):
        out += np.float32(down_filter[j]) * zp[:, j:j + 2 * t - 1:2]
    return out


def block_params(b):
    """(out_start, window_start, out_len) for block b."""
    if b == NB - 1:
        return 116 * b, T_ - 128, T_ - 116 * b
    return 116 * b, max(0, 116 * b - 6), 116


def _variant_anchor(variant):
    if variant == 'first':
        return 0, 0, 116
    if variant == 'int':
        return 192, 186, 116
    return TP - 72, TP - 128, 72


_VARIANTS = ('first', 'int', 'last')
_UOPS = ('U0', 'U1', 'G')             # fp16 weight block order
_DOPS = ('D0', 'D1')                  # bf16 weight block order


def build_weights(up_filter, down_filter):
    I = np.eye(TP, dtype=np.float32)
    Ufull = _np_up_linop(I, up_filter).T
    U0p, U1p = Ufull[0::2, :], Ufull[1::2, :]
    I2 = np.eye(2 * TP, dtype=np.float32)
    Dfull = _np_down_linop(I2, down_filter).T
    D0p, D1p = Dfull[:, 0::2], Dfull[:, 1::2]
    Gp = (Dfull.astype(np.float64) @ Ufull.astype(np.float64)).astype(np.float32)

    Ws = {}
    o = np.arange(128)
    p = np.arange(128)
    for variant in _VARIANTS:
        s, w, L = _variant_anchor(variant)
        c_pos = s - 3 + o
        x_pos = w + p
        cv = (c_pos >= 0) & (c_pos < TP)
        xv = (x_pos >= 0) & (x_pos < TP)
        for name, P in (('U0', U0p), ('U1', U1p)):
            W = np.zeros((128, 128), np.float32)
            W[np.ix_(xv, cv)] = P[np.ix_(c_pos[cv], x_pos[xv])].T
            Ws[(name, variant)] = W
        out_pos = s + o
        ov = (o < L) & (out_pos < TP)
        for name, P in (('D0', D0p), ('D1', D1p)):
            W = np.zeros((128, 128), np.float32)
            W[np.ix_(cv, ov)] = P[np.ix_(out_pos[ov], c_pos[cv])].T
            Ws[(name, variant)] = W
        W = np.zeros((128, 128), np.float32)
        W[np.ix_(xv, ov)] = Gp[np.ix_(out_pos[ov], x_pos[xv])].T
        Ws[('G', variant)] = W
    return Ws


def host_consts(alpha, beta, up_filter, down_filter):
    alpha = np.asarray(alpha, dtype=np.float32)
    beta = np.asarray(beta, dtype=np.float32)
    Ws = build_weights(np.asarray(up_filter, np.float32),
                       np.asarray(down_filter, np.float32))
    wu = np.concatenate([Ws[(op, v)] for op in _UOPS for v in _VARIANTS],
                        axis=1).astype(np.float16)          # [128, 9*128]
    wd = np.concatenate([Ws[(op, v)] for op in _DOPS for v in _VARIANTS],
                        axis=1).astype(ml_dtypes.bfloat16)  # [128, 6*128]

    arow = np.exp(np.tile(alpha, ROWS // C_)).astype(np.float32)
    invb = (1.0 / (np.exp(np.tile(beta, ROWS // C_)) + 1e-9)).astype(np.float32)
    a2 = (arow / np.float32(math.pi)).astype(np.float32)
    a2t2 = np.broadcast_to(a2[None, :], (128, ROWS)).astype(np.float16)
    invbt = np.broadcast_to(np.tile(invb, 2)[None, :],
                            (128, 2 * ROWS)).astype(ml_dtypes.bfloat16)
    identb = np.eye(128, dtype=np.float32).astype(ml_dtypes.bfloat16)
    return dict(wu=wu, wd=wd, a2t2=a2t2, invbt=invbt, identb=identb)


# ---------------------------------------------------------------- bass kernel
def _build_bass():
    nc = bacc.Bacc()
    xs = nc.dram_tensor("xs", [ROWS, T_], F16, kind="ExternalInput")
    wu_d = nc.dram_tensor("wu", [128, 9 * 128], F16, kind="ExternalInput")
    wd_d = nc.dram_tensor("wd", [128, 6 * 128], BF16, kind="ExternalInput")
    a2t2_d = nc.dram_tensor("a2t2", [128, ROWS], F16, kind="ExternalInput")
    invbt_d = nc.dram_tensor("invbt", [128, 2 * ROWS], BF16, kind="ExternalInput")
    identb_d = nc.dram_tensor("identb", [128, 128], BF16, kind="ExternalInput")
    outs = nc.dram_tensor("outs", [ROWS, T_], BF16, kind="ExternalOutput")

    PI = math.pi

    with TileContext(nc) as tc:
        with (
            tc.tile_pool(name="const", bufs=1) as pconst,
            tc.tile_pool(name="xut", bufs=3) as pxut,
            tc.tile_pool(name="xat", bufs=3) as pxat,
            tc.tile_pool(name="m", bufs=6) as pm,
            tc.tile_pool(name="s", bufs=2) as ps,
            tc.tile_pool(name="q", bufs=4) as pq,
            tc.tile_pool(name="z", bufs=2) as pz,
            tc.tile_pool(name="og", bufs=2) as pog,
            tc.tile_pool(name="r", bufs=2, space="PSUM") as pr,
            tc.tile_pool(name="pp", bufs=2, space="PSUM") as ppp,
            tc.tile_pool(name="os", bufs=2, space="PSUM") as pos,
        ):
            # ---- consts: DMA then one DVE staging tick each ----
            wu_s = pconst.tile([128, 9 * 128], F16, tag="wu_s")
            nc.sync.dma_start(out=wu_s[:], in_=wu_d[:])
            wd_s = pconst.tile([128, 6 * 128], BF16, tag="wd_s")
            nc.sync.dma_start(out=wd_s[:], in_=wd_d[:])
            a2_s = pconst.tile([128, ROWS], F16, tag="a2_s")
            nc.sync.dma_start(out=a2_s[:], in_=a2t2_d[:])
            ib_s = pconst.tile([128, 2 * ROWS], BF16, tag="ib_s")
            nc.sync.dma_start(out=ib_s[:], in_=invbt_d[:])
            id_s = pconst.tile([128, 128], BF16, tag="id_s")
            nc.sync.dma_start(out=id_s[:], in_=identb_d[:])

            wu = pconst.tile([128, 9 * 128], F16, tag="wu")
            nc.vector.tensor_copy(wu[:], wu_s[:])
            wd = pconst.tile([128, 6 * 128], BF16, tag="wd")
            nc.vector.tensor_copy(wd[:], wd_s[:])
            a2t2 = pconst.tile([128, ROWS], F16, tag="a2t2")
            nc.vector.tensor_copy(a2t2[:], a2_s[:])
            invbt = pconst.tile([128, 2 * ROWS], BF16, tag="invbt")
            nc.vector.tensor_copy(invbt[:], ib_s[:])
            identb = pconst.tile([128, 128], BF16, tag="identb")
            nc.vector.tensor_copy(identb[:], id_s[:])
            mpi = pconst.tile([128, 1], F32, tag="mpi")
            nc.vector.memset(mpi[:], 0.0)

            def wsel(op, b):
                v = 0 if b == 0 else (2 if b == NB - 1 else 1)
                if op in _UOPS:
                    i = _UOPS.index(op) * 3 + v
                    return wu[:, i * 128:(i + 1) * 128]
                i = _DOPS.index(op) * 3 + v
                return wd[:, i * 128:(i + 1) * 128]

            # PE warmup: a transpose that waits on the const staging tick
            warm = pos.tile([128, 4 * 116], BF16, tag="os")
            nc.tensor.transpose(warm[:, 0:128], identb[:], identb[:])

            xuts = {}
            xats = {}
            qs = {}
            ogs = {}

            def stage_in(b):
                s, w, L = block_params(b)
                xut = pxut.tile([128, ROWS], F16, tag="xut")
                nc.sync.dma_start_transpose(xut[:], xs[:, w:w + 128])
                xat2 = pxat.tile([128, ROWS], F16, tag="xat")
                nc.vector.tensor_tensor(out=xat2[:], in0=xut[:], in1=a2t2[:],
                                        op=mybir.AluOpType.mult)
                xuts[b] = xut
                xats[b] = xat2

            def stage_up(b):
                r01 = pr.tile([128, 2 * ROWS], F32, tag="r")
                nc.tensor.matmul(r01[:, 0:ROWS], wsel('U0', b), xats[b][:],
                                 start=True, stop=True)
                nc.tensor.matmul(r01[:, ROWS:], wsel('U1', b), xats[b][:],
                                 start=True, stop=True)
                # range reduction: rr=fp16(r); n2p=fp16(rr+3072) rounds to an
                # even integer + 3072 (fp16 spacing there is 2.0);
                # m' = (n2p-3072) - rr = round2(r) - r in [-1,1],
                # sin(pi*m') = -sin(pi*r); the sign cancels in the square.
                rr = pm.tile([128, 2 * ROWS], F16, tag="rr")
                nc.scalar.copy(rr[:], r01[:])
                n2p = pm.tile([128, 2 * ROWS], F16, tag="n2p")
                nc.vector.tensor_scalar(
                    out=n2p[:], in0=rr[:], scalar1=MAGIC16, scalar2=None,
                    op0=mybir.AluOpType.add)
                m01 = pm.tile([128, 2 * ROWS], F16, tag="m")
                nc.vector.scalar_tensor_tensor(
                    out=m01[:], in0=n2p[:], scalar=MAGIC16, in1=rr[:],
                    op0=mybir.AluOpType.subtract, op1=mybir.AluOpType.subtract)
                s01 = ps.tile([128, 2 * ROWS], BF16, tag="s")
                nc.scalar.activation(s01[:], m01[:],
                                     mybir.ActivationFunctionType.Sin,
                                     bias=mpi[:], scale=PI)
                sscl = pq.tile([128, 2 * ROWS], BF16, tag="sscl")
                nc.vector.tensor_tensor(out=sscl[:], in0=s01[:], in1=invbt[:],
                                        op=mybir.AluOpType.mult)
                q01 = pq.tile([128, 2 * ROWS], BF16, tag="q")
                nc.vector.tensor_tensor(out=q01[:], in0=s01[:], in1=sscl[:],
                                        op=mybir.AluOpType.mult)
                qs[b] = q01

            def stage_out(b):
                s, w, L = block_params(b)
                g, gi = divmod(b, GRP)
                P = ppp.tile([128, ROWS], F32, tag="p")
                nc.tensor.matmul(P[:], wsel('G', b), xuts[b][:],
                                 start=True, stop=False)
                nc.tensor.matmul(P[:], wsel('D0', b), qs[b][:, 0:ROWS],
                                 start=False, stop=False)
                nc.tensor.matmul(P[:], wsel('D1', b), qs[b][:, ROWS:],
                                 start=False, stop=True)
                zsum = pz.tile([128, ROWS], BF16, tag="z")
                nc.scalar.copy(zsum[:], P[:])
                osps = pos.tile([128, 4 * 116], BF16, tag="os")
                for c in range(4):
                    nc.tensor.transpose(osps[:, c * 116:c * 116 + L],
                                        zsum[0:L, c * 128:(c + 1) * 128],
                                        identb[0:L, 0:L])
                if gi == 0:
                    ogs[g] = pog.tile([128, 4 * GRP * 116], BF16, name="og",
                                      tag="og")
                og = ogs[g]
                ogv = og.rearrange("p (c t) -> p c t", c=4)
                nc.vector.tensor_copy(
                    ogv[:, :, gi * 116:gi * 116 + L],
                    osps.rearrange("p (c t) -> p c t", c=4)[:, :, 0:L])
                gl = GRP if g < NGRP - 1 else NB - GRP * (NGRP - 1)
                if gi == gl - 1:
                    t0 = 116 * GRP * g
                    tw = (s + L) - t0
                    nc.sync.dma_start(
                        out=outs.rearrange("(c p) t -> p c t", c=4)[:, :, t0:t0 + tw],
                        in_=ogv[:, :, 0:tw])
                    nc.vector.memset(og[0:1, 0:1], 0.0)

            for i in range(NB + 2):
                if i < NB:
                    stage_in(i)
                if 1 <= i <= NB:
                    stage_up(i - 1)
                if 2 <= i:
                    stage_out(i - 2)

    nc.compile()
    return nc


_NC_CACHE = {}


def kernel(x, alpha, beta, up_filter, down_filter):
    x = np.asarray(x, dtype=np.float32)
    consts = host_consts(alpha, beta, up_filter, down_filter)

    if 'nc' not in _NC_CACHE:
        _NC_CACHE['nc'] = _build_bass()
    nc = _NC_CACHE['nc']

    rows = x.reshape(B_ * C_, T_)
    in_maps = []
    for k in range(NCORES):
        shard = np.ascontiguousarray(rows[k * ROWS:(k + 1) * ROWS]).astype(np.float16)
        in_maps.append(dict(xs=shard, **consts))

    res = run_bass_kernel_spmd(nc, in_maps, core_ids=list(range(NCORES)),
                               **_RUN_KW)
    out = np.concatenate([np.asarray(r["outs"]).astype(np.float32)
                          for r in res.results], axis=0)
    kernel.last_result = res
    return out.reshape(B_, C_, T_)


_RUN_KW = {}
kernel.last_result = None
